# revision 9
# baseline (speedup 1.0000x reference)
"""DLinear (causal-window decomposition + dual Linear) as a single fused matmul
on 8 Trainium2 NeuronCores.

Algebra: with A the [T,T] causal-window-mean operator (banded, window=25),
    trend = x @ A^T
    out   = trend @ Tw^T + (x - trend) @ Sw^T + (tb + sb)
          = x @ (Sw + (Tw - Sw) @ A)^T + (tb + sb)
A is banded (25 nonzeros/row), so (Tw-Sw)@A folds on the host in O(T^2) via a
reversed windowed column-sum.  The device then runs one [2048,721] x [721,720]
matmul per core; the bias rides as an extra contraction row against a ones row
appended to x^T.

The per-core x shard is shipped pre-transposed and padded to 768 rows
([768, 2048], contraction dim on partitions) so the device does no PE
transposes and each column-group loads with a single 3D-AP DMA.  Data moves
as fp16 (x, W, out) with fp32 PSUM accumulation: fp16xfp16 products are
exact in fp32, end-to-end error ~5e-4 of output scale while halving DMA
traffic.  Output is upcast to fp32 on the host.
"""

import numpy as np

import concourse.bacc as bacc
import concourse.mybir as mybir
from concourse import tile
from concourse.bass_utils import run_bass_kernel_spmd

WINDOW = 25
B, NPTS, T = 32, 512, 720
U = T                     # output features
N_CORES = 8
M_TOT = B * NPTS          # 16384 rows
M_LOC = M_TOT // N_CORES  # 2048 rows per core
P = 128                   # partitions
M_TILES = M_LOC // P      # 16
KE = T + 1                # contraction incl. bias row
K_CHUNKS = [(k * P, min(P, KE - k * P)) for k in range((KE + P - 1) // P)]
NK = len(K_CHUNKS)        # 6
KP = NK * P               # 768, row-padded contraction extent
N_CHUNKS = [(0, 360), (360, 360)]  # PSUM-bank-sized slices of U
# x column-group widths for pipelined loading; first small to start PE early
GROUPS = [(0, 128), (128, 384), (512, 512), (1024, 512), (1536, 512)]

_F32 = mybir.dt.float32
_F16 = mybir.dt.float16
N_WARMUP = 8              # junk matmuls to lift the PE HAM clock-gate


def _build_nc():
    nc = bacc.Bacc("TRN2", target_bir_lowering=False, debug=False,
                   num_devices=N_CORES, enable_partition_id=False)
    xt_d = nc.dram_tensor("xt", [KP, M_LOC], _F16, kind="ExternalInput").ap()
    wt_d = nc.dram_tensor("wt", [KP, U], _F16, kind="ExternalInput").ap()
    out_d = nc.dram_tensor("out", [M_LOC, U], _F16, kind="ExternalOutput").ap()
    xt_v = xt_d.rearrange("(k p) m -> p k m", p=P)
    wt_v = wt_d.rearrange("(k p) u -> p k u", p=P)

    with tile.TileContext(nc) as tc:
        with tc.tile_pool(name="wpool", bufs=1) as wpool, \
             tc.tile_pool(name="xpool", bufs=1) as xpool, \
             tc.tile_pool(name="opool", bufs=4) as opool, \
             tc.tile_pool(name="wup", bufs=1, space="PSUM") as wup, \
             tc.tile_pool(name="accp", bufs=6, space="PSUM") as accp:

            # HAM warm-up: junk matmuls keep the PE busy while the first
            # DMAs land, so real matmuls start at the 2.4 GHz clock.
            scr = wpool.tile([P, 384], _F16, name="scr", tag="scr")
            nc.gpsimd.memset(scr[:], 0.0)
            ps_scr = wup.tile([P, 384], _F32, name="ps_scr", tag="ps_scr")
            for _ in range(N_WARMUP):
                nc.tensor.matmul(ps_scr[:], scr[:, 0:P], scr[:],
                                 start=True, stop=True)

            # Weights resident in SBUF; one 3D-AP DMA per n-half (SWDGE) so
            # the first-matmul gate is [w n0-half + x group 0].
            w_all = wpool.tile([P, NK * U], _F16, name="w_all", tag="w_all")
            w_v = w_all[:].rearrange("p (k u) -> p k u", k=NK)
            nc.gpsimd.dma_start(w_v[:, :, 0:360], wt_v[:, :, 0:360])
            nc.gpsimd.dma_start(w_v[:, :, 360:720], wt_v[:, :, 360:720])

            # x^T fully resident; one 3D-AP DMA per column group (HWDGE).
            x_all = xpool.tile([P, NK * M_LOC], _F16, name="x_all", tag="x_all")
            x_v = x_all[:].rearrange("p (k m) -> p k m", k=NK)
            for c0, cw in GROUPS:
                nc.sync.dma_start(x_v[:, :, c0:c0 + cw], xt_v[:, :, c0:c0 + cw])

            # (m, n) schedule: n-major inside group 0 so the first matmuls
            # need only the n0 half of the weights.
            plan = []
            for g, (c0, cw) in enumerate(GROUPS):
                ms = range(c0 // P, (c0 + cw) // P)
                if g == 0:
                    plan += [(m, 0) for m in ms] + [(m, 1) for m in ms]
                else:
                    plan += [(m, n) for m in ms for n in (0, 1)]

            ot_tiles = {}
            done = {}
            for m, n in plan:
                if m not in ot_tiles:
                    ot_tiles[m] = opool.tile([P, U], _F16, name="ot")
                n0, nw = N_CHUNKS[n]
                acc = accp.tile([P, 512], _F32, name="acc", tag="acc")
                for k, (k0, kc) in enumerate(K_CHUNKS):
                    nc.tensor.matmul(
                        acc[:, 0:nw],
                        x_v[0:kc, k, m * P:(m + 1) * P],
                        w_v[0:kc, k, n0:n0 + nw],
                        start=(k == 0), stop=(k == NK - 1))
                nc.vector.tensor_copy(ot_tiles[m][:, n0:n0 + nw], acc[:, 0:nw])
                done[m] = done.get(m, 0) + 1
                if done[m] == 2:
                    nc.scalar.dma_start(out_d[m * P:(m + 1) * P, :],
                                        ot_tiles.pop(m)[:])

    nc.compile()
    return nc


def _fold_weights(trend_w, seasonal_w, trend_b, seasonal_b):
    """W = seasonal_w + (trend_w - seasonal_w) @ A via the banded structure of
    A; returns [KE, U] = [W^T; b] ready for the device."""
    counts = np.minimum(np.arange(T) + 1, WINDOW).astype(np.float64)
    G = (trend_w.astype(np.float64) - seasonal_w.astype(np.float64)) / counts[None, :]
    M = np.zeros_like(G)
    for d in range(WINDOW):
        M[:, :T - d] += G[:, d:]
    W = seasonal_w.astype(np.float64) + M
    b = trend_b.astype(np.float64) + seasonal_b.astype(np.float64)
    wt_ext = np.empty((KE, U), np.float32)
    wt_ext[:T, :] = W.T.astype(np.float32)
    wt_ext[T, :] = b.astype(np.float32)
    return wt_ext


_NC_CACHE = {}
RUN_KWARGS = {}   # test harness may set {"trace": True}
LAST_RESULTS = None


def kernel(x, trend_w, trend_b, seasonal_w, seasonal_b):
    global LAST_RESULTS
    wt_ext = _fold_weights(trend_w, seasonal_w, trend_b, seasonal_b)

    # Pre-transposed, ones-row-extended, 768-row-padded fp16 shards.
    x2d = np.asarray(x, dtype=np.float32).reshape(M_TOT, T)
    xt_all = np.zeros((KP, M_TOT), np.float16)
    xt_all[:T] = x2d.T.astype(np.float16)
    xt_all[T] = 1.0
    xt_cores = np.ascontiguousarray(
        xt_all.reshape(KP, N_CORES, M_LOC).transpose(1, 0, 2))

    wt16 = np.zeros((KP, U), np.float16)
    wt16[:KE] = wt_ext.astype(np.float16)

    if "nc" not in _NC_CACHE:
        _NC_CACHE["nc"] = _build_nc()
    nc = _NC_CACHE["nc"]

    in_maps = [{"xt": xt_cores[i], "wt": wt16} for i in range(N_CORES)]
    res = run_bass_kernel_spmd(nc, in_maps, core_ids=list(range(N_CORES)),
                               **RUN_KWARGS)
    LAST_RESULTS = res
    out = np.concatenate([r["out"] for r in res.results], axis=0)
    return out.astype(np.float32).reshape(B, NPTS, U)


# revision 11
# speedup vs baseline: 1.0919x; 1.0919x over previous
"""DLinear (causal-window decomposition + dual Linear) as a single fused matmul
on 8 Trainium2 NeuronCores.

Algebra: with A the [T,T] causal-window-mean operator (banded, window=25),
    trend = x @ A^T
    out   = trend @ Tw^T + (x - trend) @ Sw^T + (tb + sb)
          = x @ (Sw + (Tw - Sw) @ A)^T + (tb + sb)
A is banded (25 nonzeros/row), so (Tw-Sw)@A folds on the host in O(T^2) via a
reversed windowed column-sum.  The device then runs one [2048,721] x [721,720]
matmul per core; the bias rides as an extra contraction row against a ones row
appended to x^T.

The per-core x shard is shipped pre-transposed and padded to 768 rows
([768, 2048], contraction dim on partitions) so the device does no PE
transposes and each column-group loads with a single 3D-AP DMA.  Data moves
as fp16 (x, W, out) with fp32 PSUM accumulation: fp16xfp16 products are
exact in fp32, end-to-end error ~5e-4 of output scale while halving DMA
traffic.  Output is upcast to fp32 on the host.
"""

import numpy as np

import concourse.bacc as bacc
import concourse.mybir as mybir
from concourse import tile
from concourse.bass_utils import run_bass_kernel_spmd

WINDOW = 25
B, NPTS, T = 32, 512, 720
U = T                     # output features
N_CORES = 8
M_TOT = B * NPTS          # 16384 rows
M_LOC = M_TOT // N_CORES  # 2048 rows per core
P = 128                   # partitions
M_TILES = M_LOC // P      # 16
KE = T + 1                # contraction incl. bias row
K_CHUNKS = [(k * P, min(P, KE - k * P)) for k in range((KE + P - 1) // P)]
NK = len(K_CHUNKS)        # 6
KP = NK * P               # 768, row-padded contraction extent
N_CHUNKS = [(0, 360), (360, 360)]  # PSUM-bank-sized slices of U
# x column-group widths for pipelined loading; first small to start PE early
GROUPS = [(0, 128), (128, 256), (384, 384), (768, 640), (1408, 640)]

_F32 = mybir.dt.float32
_F16 = mybir.dt.float16
N_WARMUP = 9              # junk matmuls to lift the PE HAM clock-gate


def _build_nc():
    nc = bacc.Bacc("TRN2", target_bir_lowering=False, debug=False,
                   num_devices=N_CORES, enable_partition_id=False)
    xt_d = nc.dram_tensor("xt", [KP, M_LOC], _F16, kind="ExternalInput").ap()
    wt_d = nc.dram_tensor("wt", [KP, U], _F16, kind="ExternalInput").ap()
    out_d = nc.dram_tensor("out", [M_LOC, U], _F16, kind="ExternalOutput").ap()
    xt_v = xt_d.rearrange("(k p) m -> p k m", p=P)
    wt_v = wt_d.rearrange("(k p) u -> p k u", p=P)

    with tile.TileContext(nc) as tc:
        with tc.tile_pool(name="wpool", bufs=1) as wpool, \
             tc.tile_pool(name="xpool", bufs=1) as xpool, \
             tc.tile_pool(name="opool", bufs=4) as opool, \
             tc.tile_pool(name="wup", bufs=1, space="PSUM") as wup, \
             tc.tile_pool(name="accp", bufs=6, space="PSUM") as accp:

            # HAM warm-up: junk matmuls keep the PE busy while the first
            # DMAs land, so real matmuls start at the 2.4 GHz clock.
            scr = wpool.tile([P, 384], _F16, name="scr", tag="scr")
            nc.gpsimd.memset(scr[:], 0.0)
            ps_scr = wup.tile([P, 384], _F32, name="ps_scr", tag="ps_scr")
            for _ in range(N_WARMUP):
                nc.tensor.matmul(ps_scr[:], scr[:, 0:P], scr[:],
                                 start=True, stop=True)

            # Inputs all on the HWDGE sync queue, in gate order:
            # w-n0 half, x group 0, w-n1 half, remaining x groups.
            w_all = wpool.tile([P, NK * U], _F16, name="w_all", tag="w_all")
            w_v = w_all[:].rearrange("p (k u) -> p k u", k=NK)
            x_all = xpool.tile([P, NK * M_LOC], _F16, name="x_all", tag="x_all")
            x_v = x_all[:].rearrange("p (k m) -> p k m", k=NK)

            nc.sync.dma_start(w_v[:, :, 0:360], wt_v[:, :, 0:360])
            c0, cw = GROUPS[0]
            nc.sync.dma_start(x_v[:, :, c0:c0 + cw], xt_v[:, :, c0:c0 + cw])
            nc.sync.dma_start(w_v[:, :, 360:720], wt_v[:, :, 360:720])
            for c0, cw in GROUPS[1:]:
                nc.sync.dma_start(x_v[:, :, c0:c0 + cw], xt_v[:, :, c0:c0 + cw])

            # (m, n) schedule: n-major inside group 0 so the first matmuls
            # need only the n0 half of the weights.
            plan = []
            for g, (c0, cw) in enumerate(GROUPS):
                ms = range(c0 // P, (c0 + cw) // P)
                if g == 0:
                    plan += [(m, 0) for m in ms] + [(m, 1) for m in ms]
                else:
                    plan += [(m, n) for m in ms for n in (0, 1)]

            ot_tiles = {}
            done = {}
            for m, n in plan:
                if m not in ot_tiles:
                    ot_tiles[m] = opool.tile([P, U], _F16, name="ot")
                n0, nw = N_CHUNKS[n]
                acc = accp.tile([P, 512], _F32, name="acc", tag="acc")
                for k, (k0, kc) in enumerate(K_CHUNKS):
                    nc.tensor.matmul(
                        acc[:, 0:nw],
                        x_v[0:kc, k, m * P:(m + 1) * P],
                        w_v[0:kc, k, n0:n0 + nw],
                        start=(k == 0), stop=(k == NK - 1))
                nc.vector.tensor_copy(ot_tiles[m][:, n0:n0 + nw], acc[:, 0:nw])
                done[m] = done.get(m, 0) + 1
                if done[m] == 2:
                    nc.scalar.dma_start(out_d[m * P:(m + 1) * P, :],
                                        ot_tiles.pop(m)[:])

    nc.compile()
    return nc


def _fold_weights(trend_w, seasonal_w, trend_b, seasonal_b):
    """W = seasonal_w + (trend_w - seasonal_w) @ A via the banded structure of
    A; returns [KE, U] = [W^T; b] ready for the device."""
    counts = np.minimum(np.arange(T) + 1, WINDOW).astype(np.float64)
    G = (trend_w.astype(np.float64) - seasonal_w.astype(np.float64)) / counts[None, :]
    M = np.zeros_like(G)
    for d in range(WINDOW):
        M[:, :T - d] += G[:, d:]
    W = seasonal_w.astype(np.float64) + M
    b = trend_b.astype(np.float64) + seasonal_b.astype(np.float64)
    wt_ext = np.empty((KE, U), np.float32)
    wt_ext[:T, :] = W.T.astype(np.float32)
    wt_ext[T, :] = b.astype(np.float32)
    return wt_ext


_NC_CACHE = {}
RUN_KWARGS = {}   # test harness may set {"trace": True}
LAST_RESULTS = None


def kernel(x, trend_w, trend_b, seasonal_w, seasonal_b):
    global LAST_RESULTS
    wt_ext = _fold_weights(trend_w, seasonal_w, trend_b, seasonal_b)

    # Pre-transposed, ones-row-extended, 768-row-padded fp16 shards.
    x2d = np.asarray(x, dtype=np.float32).reshape(M_TOT, T)
    xt_all = np.zeros((KP, M_TOT), np.float16)
    xt_all[:T] = x2d.T.astype(np.float16)
    xt_all[T] = 1.0
    xt_cores = np.ascontiguousarray(
        xt_all.reshape(KP, N_CORES, M_LOC).transpose(1, 0, 2))

    wt16 = np.zeros((KP, U), np.float16)
    wt16[:KE] = wt_ext.astype(np.float16)

    if "nc" not in _NC_CACHE:
        _NC_CACHE["nc"] = _build_nc()
    nc = _NC_CACHE["nc"]

    in_maps = [{"xt": xt_cores[i], "wt": wt16} for i in range(N_CORES)]
    res = run_bass_kernel_spmd(nc, in_maps, core_ids=list(range(N_CORES)),
                               **RUN_KWARGS)
    LAST_RESULTS = res
    out = np.concatenate([r["out"] for r in res.results], axis=0)
    return out.astype(np.float32).reshape(B, NPTS, U)


# revision 13
# speedup vs baseline: 1.1197x; 1.0255x over previous
"""DLinear (causal-window decomposition + dual Linear) as a single fused matmul
on 8 Trainium2 NeuronCores.

Algebra: with A the [T,T] causal-window-mean operator (banded, window=25),
    trend = x @ A^T
    out   = trend @ Tw^T + (x - trend) @ Sw^T + (tb + sb)
          = x @ (Sw + (Tw - Sw) @ A)^T + (tb + sb)
A is banded (25 nonzeros/row), so (Tw-Sw)@A folds on the host in O(T^2) via a
reversed windowed column-sum.  The device then runs one [2048,721] x [721,720]
matmul per core; the bias rides as an extra contraction row against a ones row
appended to x^T.

The per-core x shard is shipped pre-transposed and padded to 768 rows
([768, 2048], contraction dim on partitions) so the device does no PE
transposes and each column-group loads with a single 3D-AP DMA.  Data moves
as fp16 (x, W, out) with fp32 PSUM accumulation: fp16xfp16 products are
exact in fp32, end-to-end error ~5e-4 of output scale while halving DMA
traffic.  Output is upcast to fp32 on the host.
"""

import numpy as np

import concourse.bacc as bacc
import concourse.mybir as mybir
from concourse import tile
from concourse.bass_utils import run_bass_kernel_spmd

WINDOW = 25
B, NPTS, T = 32, 512, 720
U = T                     # output features
N_CORES = 8
M_TOT = B * NPTS          # 16384 rows
M_LOC = M_TOT // N_CORES  # 2048 rows per core
P = 128                   # partitions
M_TILES = M_LOC // P      # 16
KE = T + 1                # contraction incl. bias row
K_CHUNKS = [(k * P, min(P, KE - k * P)) for k in range((KE + P - 1) // P)]
NK = len(K_CHUNKS)        # 6
KP = NK * P               # 768, row-padded contraction extent
N_CHUNKS = [(0, 360), (360, 360)]  # PSUM-bank-sized slices of U
# x column-group widths for pipelined loading; first small to start PE early
GROUPS = [(0, 128), (128, 256), (384, 384), (768, 640), (1408, 640)]

_F32 = mybir.dt.float32
_F16 = mybir.dt.float16
N_WARMUP = 14             # junk matmuls to lift the PE HAM clock-gate
N_FILLER = 2              # junk matmuls after each early unit (DMA-stall gap fill)
FILLER_UNITS = 5          # how many leading plan units get filler


def _build_nc():
    nc = bacc.Bacc("TRN2", target_bir_lowering=False, debug=False,
                   num_devices=N_CORES, enable_partition_id=False)
    xt_d = nc.dram_tensor("xt", [KP, M_LOC], _F16, kind="ExternalInput").ap()
    wt_d = nc.dram_tensor("wt", [KP, U], _F16, kind="ExternalInput").ap()
    out_d = nc.dram_tensor("out", [M_LOC, U], _F16, kind="ExternalOutput").ap()
    xt_v = xt_d.rearrange("(k p) m -> p k m", p=P)
    wt_v = wt_d.rearrange("(k p) u -> p k u", p=P)

    with tile.TileContext(nc) as tc:
        with tc.tile_pool(name="wpool", bufs=1) as wpool, \
             tc.tile_pool(name="xpool", bufs=1) as xpool, \
             tc.tile_pool(name="opool", bufs=4) as opool, \
             tc.tile_pool(name="wup", bufs=1, space="PSUM") as wup, \
             tc.tile_pool(name="accp", bufs=6, space="PSUM") as accp:

            # HAM warm-up: junk matmuls keep the PE busy while the first
            # DMAs land, so real matmuls start at the 2.4 GHz clock.
            scr = wpool.tile([P, 384], _F16, name="scr", tag="scr")
            nc.gpsimd.memset(scr[:], 0.0)
            ps_scr = wup.tile([P, 384], _F32, name="ps_scr", tag="ps_scr")
            for _ in range(N_WARMUP):
                nc.tensor.matmul(ps_scr[:], scr[:, 0:P], scr[:],
                                 start=True, stop=True)

            # Inputs all on the HWDGE sync queue, in gate order:
            # w-n0 half, x group 0, w-n1 half, remaining x groups.
            w_all = wpool.tile([P, NK * U], _F16, name="w_all", tag="w_all")
            w_v = w_all[:].rearrange("p (k u) -> p k u", k=NK)
            x_all = xpool.tile([P, NK * M_LOC], _F16, name="x_all", tag="x_all")
            x_v = x_all[:].rearrange("p (k m) -> p k m", k=NK)

            nc.sync.dma_start(w_v[:, :, 0:360], wt_v[:, :, 0:360])
            c0, cw = GROUPS[0]
            nc.sync.dma_start(x_v[:, :, c0:c0 + cw], xt_v[:, :, c0:c0 + cw])
            nc.sync.dma_start(w_v[:, :, 360:720], wt_v[:, :, 360:720])
            for c0, cw in GROUPS[1:]:
                nc.sync.dma_start(x_v[:, :, c0:c0 + cw], xt_v[:, :, c0:c0 + cw])

            # (m, n) schedule: n-major inside group 0 so the first matmuls
            # need only the n0 half of the weights.
            plan = []
            for g, (c0, cw) in enumerate(GROUPS):
                ms = range(c0 // P, (c0 + cw) // P)
                if g == 0:
                    plan += [(m, 0) for m in ms] + [(m, 1) for m in ms]
                else:
                    plan += [(m, n) for m in ms for n in (0, 1)]

            ot_tiles = {}
            done = {}
            last_m = plan[-1][0]
            for u_idx, (m, n) in enumerate(plan):
                if m not in ot_tiles:
                    ot_tiles[m] = opool.tile([P, U], _F16, name="ot")
                n0, nw = N_CHUNKS[n]
                acc = accp.tile([P, 512], _F32, name="acc", tag="acc")
                for k, (k0, kc) in enumerate(K_CHUNKS):
                    nc.tensor.matmul(
                        acc[:, 0:nw],
                        x_v[0:kc, k, m * P:(m + 1) * P],
                        w_v[0:kc, k, n0:n0 + nw],
                        start=(k == 0), stop=(k == NK - 1))
                if u_idx < FILLER_UNITS:
                    for _ in range(N_FILLER):
                        nc.tensor.matmul(ps_scr[:], scr[:, 0:P], scr[:],
                                         start=True, stop=True)
                nc.vector.tensor_copy(ot_tiles[m][:, n0:n0 + nw], acc[:, 0:nw])
                done[m] = done.get(m, 0) + 1
                if m == last_m:
                    # split the final tile's stores so the first half's DMA
                    # overlaps the second half's compute
                    nc.scalar.dma_start(out_d[m * P:(m + 1) * P, n0:n0 + nw],
                                        ot_tiles[m][:, n0:n0 + nw])
                elif done[m] == 2:
                    nc.scalar.dma_start(out_d[m * P:(m + 1) * P, :],
                                        ot_tiles.pop(m)[:])

    nc.compile()
    return nc


def _fold_weights(trend_w, seasonal_w, trend_b, seasonal_b):
    """W = seasonal_w + (trend_w - seasonal_w) @ A via the banded structure of
    A; returns [KE, U] = [W^T; b] ready for the device."""
    counts = np.minimum(np.arange(T) + 1, WINDOW).astype(np.float64)
    G = (trend_w.astype(np.float64) - seasonal_w.astype(np.float64)) / counts[None, :]
    M = np.zeros_like(G)
    for d in range(WINDOW):
        M[:, :T - d] += G[:, d:]
    W = seasonal_w.astype(np.float64) + M
    b = trend_b.astype(np.float64) + seasonal_b.astype(np.float64)
    wt_ext = np.empty((KE, U), np.float32)
    wt_ext[:T, :] = W.T.astype(np.float32)
    wt_ext[T, :] = b.astype(np.float32)
    return wt_ext


_NC_CACHE = {}
RUN_KWARGS = {}   # test harness may set {"trace": True}
LAST_RESULTS = None


def kernel(x, trend_w, trend_b, seasonal_w, seasonal_b):
    global LAST_RESULTS
    wt_ext = _fold_weights(trend_w, seasonal_w, trend_b, seasonal_b)

    # Pre-transposed, ones-row-extended, 768-row-padded fp16 shards.
    x2d = np.asarray(x, dtype=np.float32).reshape(M_TOT, T)
    xt_all = np.zeros((KP, M_TOT), np.float16)
    xt_all[:T] = x2d.T.astype(np.float16)
    xt_all[T] = 1.0
    xt_cores = np.ascontiguousarray(
        xt_all.reshape(KP, N_CORES, M_LOC).transpose(1, 0, 2))

    wt16 = np.zeros((KP, U), np.float16)
    wt16[:KE] = wt_ext.astype(np.float16)

    if "nc" not in _NC_CACHE:
        _NC_CACHE["nc"] = _build_nc()
    nc = _NC_CACHE["nc"]

    in_maps = [{"xt": xt_cores[i], "wt": wt16} for i in range(N_CORES)]
    res = run_bass_kernel_spmd(nc, in_maps, core_ids=list(range(N_CORES)),
                               **RUN_KWARGS)
    LAST_RESULTS = res
    out = np.concatenate([r["out"] for r in res.results], axis=0)
    return out.astype(np.float32).reshape(B, NPTS, U)


# revision 14
# speedup vs baseline: 1.1338x; 1.0126x over previous
"""DLinear (causal-window decomposition + dual Linear) as a single fused matmul
on 8 Trainium2 NeuronCores.

Algebra: with A the [T,T] causal-window-mean operator (banded, window=25),
    trend = x @ A^T
    out   = trend @ Tw^T + (x - trend) @ Sw^T + (tb + sb)
          = x @ (Sw + (Tw - Sw) @ A)^T + (tb + sb)
A is banded (25 nonzeros/row), so (Tw-Sw)@A folds on the host in O(T^2) via a
reversed windowed column-sum.  The device then runs one [2048,721] x [721,720]
matmul per core; the bias rides as an extra contraction row against a ones row
appended to x^T.

The per-core x shard is shipped pre-transposed and padded to 768 rows
([768, 2048], contraction dim on partitions) so the device does no PE
transposes and each column-group loads with a single 3D-AP DMA.  Data moves
as fp16 (x, W, out) with fp32 PSUM accumulation: fp16xfp16 products are
exact in fp32, end-to-end error ~5e-4 of output scale while halving DMA
traffic.  Output is upcast to fp32 on the host.
"""

import numpy as np

import concourse.bacc as bacc
import concourse.mybir as mybir
from concourse import tile
from concourse.bass_utils import run_bass_kernel_spmd

WINDOW = 25
B, NPTS, T = 32, 512, 720
U = T                     # output features
N_CORES = 8
M_TOT = B * NPTS          # 16384 rows
M_LOC = M_TOT // N_CORES  # 2048 rows per core
P = 128                   # partitions
M_TILES = M_LOC // P      # 16
KE = T + 1                # contraction incl. bias row
K_CHUNKS = [(k * P, min(P, KE - k * P)) for k in range((KE + P - 1) // P)]
NK = len(K_CHUNKS)        # 6
KP = NK * P               # 768, row-padded contraction extent
N_CHUNKS = [(0, 360), (360, 360)]  # PSUM-bank-sized slices of U
# x column-group widths for pipelined loading; first small to start PE early
GROUPS = [(0, 128), (128, 256), (384, 384), (768, 640), (1408, 640)]

_F32 = mybir.dt.float32
_F16 = mybir.dt.float16
N_WARMUP = 12             # junk matmuls to lift the PE HAM clock-gate
N_FILLER = 1              # junk matmuls after each early unit (DMA-stall gap fill)
FILLER_UNITS = 4          # how many leading plan units get filler


def _build_nc():
    nc = bacc.Bacc("TRN2", target_bir_lowering=False, debug=False,
                   num_devices=N_CORES, enable_partition_id=False)
    xt_d = nc.dram_tensor("xt", [KP, M_LOC], _F16, kind="ExternalInput").ap()
    wt_d = nc.dram_tensor("wt", [KP, U], _F16, kind="ExternalInput").ap()
    out_d = nc.dram_tensor("out", [M_LOC, U], _F16, kind="ExternalOutput").ap()
    xt_v = xt_d.rearrange("(k p) m -> p k m", p=P)
    wt_v = wt_d.rearrange("(k p) u -> p k u", p=P)

    with tile.TileContext(nc) as tc:
        with tc.tile_pool(name="wpool", bufs=1) as wpool, \
             tc.tile_pool(name="xpool", bufs=1) as xpool, \
             tc.tile_pool(name="opool", bufs=4) as opool, \
             tc.tile_pool(name="wup", bufs=1, space="PSUM") as wup, \
             tc.tile_pool(name="accp", bufs=6, space="PSUM") as accp:

            # HAM warm-up: junk matmuls keep the PE busy while the first
            # DMAs land, so real matmuls start at the 2.4 GHz clock.
            scr = wpool.tile([P, 384], _F16, name="scr", tag="scr")
            nc.gpsimd.memset(scr[:], 0.0)
            ps_scr = wup.tile([P, 384], _F32, name="ps_scr", tag="ps_scr")
            for _ in range(N_WARMUP):
                nc.tensor.matmul(ps_scr[:], scr[:, 0:P], scr[:],
                                 start=True, stop=True)

            # Inputs all on the HWDGE sync queue, in gate order:
            # w-n0 half, x group 0, w-n1 half, remaining x groups.
            w_all = wpool.tile([P, NK * U], _F16, name="w_all", tag="w_all")
            w_v = w_all[:].rearrange("p (k u) -> p k u", k=NK)
            x_all = xpool.tile([P, NK * M_LOC], _F16, name="x_all", tag="x_all")
            x_v = x_all[:].rearrange("p (k m) -> p k m", k=NK)

            nc.sync.dma_start(w_v[:, :, 0:360], wt_v[:, :, 0:360])
            c0, cw = GROUPS[0]
            nc.sync.dma_start(x_v[:, :, c0:c0 + cw], xt_v[:, :, c0:c0 + cw])
            nc.sync.dma_start(w_v[:, :, 360:720], wt_v[:, :, 360:720])
            for c0, cw in GROUPS[1:]:
                nc.sync.dma_start(x_v[:, :, c0:c0 + cw], xt_v[:, :, c0:c0 + cw])

            # (m, n) schedule: n-major inside group 0 so the first matmuls
            # need only the n0 half of the weights.
            plan = []
            for g, (c0, cw) in enumerate(GROUPS):
                ms = range(c0 // P, (c0 + cw) // P)
                if g == 0:
                    plan += [(m, 0) for m in ms] + [(m, 1) for m in ms]
                else:
                    plan += [(m, n) for m in ms for n in (0, 1)]

            ot_tiles = {}
            done = {}
            last_m = plan[-1][0]
            for u_idx, (m, n) in enumerate(plan):
                if m not in ot_tiles:
                    ot_tiles[m] = opool.tile([P, U], _F16, name="ot")
                n0, nw = N_CHUNKS[n]
                acc = accp.tile([P, 512], _F32, name="acc", tag="acc")
                for k, (k0, kc) in enumerate(K_CHUNKS):
                    nc.tensor.matmul(
                        acc[:, 0:nw],
                        x_v[0:kc, k, m * P:(m + 1) * P],
                        w_v[0:kc, k, n0:n0 + nw],
                        start=(k == 0), stop=(k == NK - 1))
                if u_idx < FILLER_UNITS:
                    for _ in range(N_FILLER):
                        nc.tensor.matmul(ps_scr[:], scr[:, 0:P], scr[:],
                                         start=True, stop=True)
                nc.vector.tensor_copy(ot_tiles[m][:, n0:n0 + nw], acc[:, 0:nw])
                done[m] = done.get(m, 0) + 1
                if m == last_m:
                    # split the final tile's stores so the first half's DMA
                    # overlaps the second half's compute
                    nc.scalar.dma_start(out_d[m * P:(m + 1) * P, n0:n0 + nw],
                                        ot_tiles[m][:, n0:n0 + nw])
                elif done[m] == 2:
                    nc.scalar.dma_start(out_d[m * P:(m + 1) * P, :],
                                        ot_tiles.pop(m)[:])

    nc.compile()
    return nc


def _fold_weights(trend_w, seasonal_w, trend_b, seasonal_b):
    """W = seasonal_w + (trend_w - seasonal_w) @ A via the banded structure of
    A; returns [KE, U] = [W^T; b] ready for the device."""
    counts = np.minimum(np.arange(T) + 1, WINDOW).astype(np.float64)
    G = (trend_w.astype(np.float64) - seasonal_w.astype(np.float64)) / counts[None, :]
    M = np.zeros_like(G)
    for d in range(WINDOW):
        M[:, :T - d] += G[:, d:]
    W = seasonal_w.astype(np.float64) + M
    b = trend_b.astype(np.float64) + seasonal_b.astype(np.float64)
    wt_ext = np.empty((KE, U), np.float32)
    wt_ext[:T, :] = W.T.astype(np.float32)
    wt_ext[T, :] = b.astype(np.float32)
    return wt_ext


_NC_CACHE = {}
RUN_KWARGS = {}   # test harness may set {"trace": True}
LAST_RESULTS = None


def kernel(x, trend_w, trend_b, seasonal_w, seasonal_b):
    global LAST_RESULTS
    wt_ext = _fold_weights(trend_w, seasonal_w, trend_b, seasonal_b)

    # Pre-transposed, ones-row-extended, 768-row-padded fp16 shards.
    x2d = np.asarray(x, dtype=np.float32).reshape(M_TOT, T)
    xt_all = np.zeros((KP, M_TOT), np.float16)
    xt_all[:T] = x2d.T.astype(np.float16)
    xt_all[T] = 1.0
    xt_cores = np.ascontiguousarray(
        xt_all.reshape(KP, N_CORES, M_LOC).transpose(1, 0, 2))

    wt16 = np.zeros((KP, U), np.float16)
    wt16[:KE] = wt_ext.astype(np.float16)

    if "nc" not in _NC_CACHE:
        _NC_CACHE["nc"] = _build_nc()
    nc = _NC_CACHE["nc"]

    in_maps = [{"xt": xt_cores[i], "wt": wt16} for i in range(N_CORES)]
    res = run_bass_kernel_spmd(nc, in_maps, core_ids=list(range(N_CORES)),
                               **RUN_KWARGS)
    LAST_RESULTS = res
    out = np.concatenate([r["out"] for r in res.results], axis=0)
    return out.astype(np.float32).reshape(B, NPTS, U)
